# revision 9
# baseline (speedup 1.0000x reference)
"""GCN encoder (2x GCNConv + linear projection, relu) on 8 Trainium2 cores.

Self-contained: hardcodes the problem shapes (N=50000, E=800000, C=128,
OUT_C=64) and the sharding strategy.  Host side does structural prep only
(edge partitioning/sorting/padding, index-list construction); all FP math
(matmuls, rsqrt, scaling, aggregation, bias, relu) runs on device.

Math identity used on device, per GCNConv layer:
    g = dinv * (x @ W.T)          (dinv = rsqrt(indeg+1), per node)
    out[d] = relu(dinv[d] * (sum_{e: dst=d} g[src_e] + g[d]) + b)
The self-contribution g[d] is folded in as an explicit (d -> d) edge, so the
aggregation is a pure unweighted gather + segment-sum over a fixed edge list.

Device mapping per core:
  - nodes sharded by contiguous range (6250/core, padded to 6272)
  - edges partitioned by dst owner, sorted by (dst window of 128, src half)
  - gather: gpsimd dma_gather (512B rows) from a replicated DRAM table
  - segment-sum: per-128-edge selection-matrix (iota is_equal dst_rel)
    matmul accumulating into a [128 dst x 128 feat] PSUM tile per window
  - layer boundary: AllGather of the locally computed scaled table G2
"""

import sys
import numpy as np

for _p in ("/opt/trn_rl_repo",):
    if _p not in sys.path:
        sys.path.append(_p)

import concourse.bacc as bacc
import concourse.tile as tile
from concourse import bass, mybir, bass_utils

F32 = mybir.dt.float32
I16 = mybir.dt.int16
AF = mybir.ActivationFunctionType
ALU = mybir.AluOpType


class Cfg:
    def __init__(self, n_nodes, n_edges, cores=8, in_c=128, hid_c=128, out_c=64):
        assert in_c == 128 and hid_c == 128
        self.N, self.E, self.CORES = n_nodes, n_edges, cores
        self.C, self.OUT_C = in_c, out_c
        assert n_nodes % cores == 0
        self.S = n_nodes // cores                       # real nodes per shard
        self.SP = -(-self.S // 128) * 128               # padded shard rows
        assert self.SP > self.S, "need pad rows in each shard for zero rows"
        self.NPAD = self.SP * cores                     # padded table rows
        assert self.NPAD % 256 == 0
        self.HALF = self.NPAD // 2                      # int16 table split
        assert self.HALF % self.SP == 0
        assert self.HALF < 32768
        self.NW = self.SP // 128                        # windows per core
        self.NT = self.NPAD // 128                      # node tiles total


CFG = Cfg(50000, 800000)


def _wrap16(a):
    """[L] -> [128, L/16] int16 idx layout for dma_gather (16-wrap, 8x repl)."""
    assert a.size % 16 == 0
    w = a.reshape(-1, 16).T.astype(np.int16)
    return np.ascontiguousarray(np.tile(w, (8, 1)))


def _host_prep(cfg, x, edge_index):
    """Build per-core device inputs + the compile-time chunk schedule."""
    N, E, C = cfg.N, cfg.E, cfg.C
    S, SP, NPAD, HALF, NW, CORES = cfg.S, cfg.SP, cfg.NPAD, cfg.HALF, cfg.NW, cfg.CORES

    src = np.asarray(edge_index[0]).astype(np.int64)
    dst = np.asarray(edge_index[1]).astype(np.int64)
    deg = np.bincount(dst, minlength=N).astype(np.float32) + 1.0

    nodes = np.arange(N, dtype=np.int64)
    s_all = np.concatenate([src, nodes])        # self loops folded as edges
    d_all = np.concatenate([dst, nodes])

    owner = d_all // S
    loc = d_all - owner * S
    srcp = (s_all // S) * SP + (s_all % S)      # padded global src id
    win = loc // 128
    rel = (loc % 128).astype(np.float32)
    hB = srcp >= HALF

    # counts[core, window, half]
    key = (owner * NW + win) * 2 + hB
    counts = np.bincount(key, minlength=CORES * NW * 2).reshape(CORES, NW, 2)
    capA = -(-counts[:, :, 0].max(axis=0) // 128)       # chunks per window
    capB = -(-counts[:, :, 1].max(axis=0) // 128)

    # degree tensors (pad rows get deg=1)
    degp = np.ones(NPAD, np.float32)
    realpos = (nodes // S) * SP + (nodes % S)
    degp[realpos] = deg
    degt = np.ascontiguousarray(degp.reshape(-1, 128).T)          # [128, NT]

    xpad = np.zeros((NPAD, C), np.float32)
    xpad[realpos] = np.asarray(x, np.float32)
    xt = np.ascontiguousarray(xpad.T)                             # [128, NPAD]

    ZROW = S  # local-to-half id of a guaranteed zero pad row (both halves)

    per_core = []
    for c in range(CORES):
        m = owner == c
        cw, cr, cs, ch = win[m], rel[m], srcp[m], hB[m]
        order = np.lexsort((ch, cw))
        cw, cr, cs, ch = cw[order], cr[order], cs[order], ch[order]
        # boundaries: searchsorted on combined key (w, half)
        k = cw * 2 + ch
        ia_parts, ib_parts, rel_parts = [], [], []
        for wi in range(NW):
            for half, cap in ((0, capA[wi]), (1, capB[wi])):
                lo = np.searchsorted(k, wi * 2 + half, "left")
                hi = np.searchsorted(k, wi * 2 + half, "right")
                n = hi - lo
                pad = cap * 128 - n
                assert pad >= 0
                iv = cs[lo:hi] - (HALF if half else 0)
                iv = np.concatenate([iv, np.full(pad, ZROW, np.int64)])
                rv = np.concatenate([cr[lo:hi], np.full(pad, -1.0, np.float32)])
                (ib_parts if half else ia_parts).append(iv)
                rel_parts.append(rv)
        idxa = np.concatenate(ia_parts) if ia_parts else np.zeros(0, np.int64)
        idxb = np.concatenate(ib_parts) if ib_parts else np.zeros(0, np.int64)
        rel_all = np.concatenate(rel_parts).astype(np.float32)
        relT = np.ascontiguousarray(rel_all.reshape(-1, 128).T)   # [128, nchunk]
        degl = np.ascontiguousarray(
            degp[c * SP:(c + 1) * SP].reshape(NW, 128).T)          # [128, NW]
        per_core.append(dict(
            idxa=_wrap16(idxa), idxb=_wrap16(idxb), rel=relT, degl=degl))

    sched = dict(capA=[int(v) for v in capA], capB=[int(v) for v in capB])
    shared = dict(xt=xt, degt=degt)
    return sched, shared, per_core


def _build_nc(cfg, sched):
    N, C, OUT_C = cfg.N, cfg.C, cfg.OUT_C
    SP, NPAD, HALF, NW, NT, CORES = (cfg.SP, cfg.NPAD, cfg.HALF, cfg.NW,
                                     cfg.NT, cfg.CORES)
    capA, capB = sched["capA"], sched["capB"]
    nchunk = sum(capA) + sum(capB)
    la16 = sum(capA) * 128 // 16
    lb16 = sum(capB) * 128 // 16

    nc = bacc.Bacc("TRN2", target_bir_lowering=False, debug=False,
                   enable_asserts=False, num_devices=CORES)

    din = {}
    def inp(name, shape, dt=F32):
        din[name] = nc.dram_tensor(name, shape, dt, kind="ExternalInput").ap()
        return din[name]

    xt_d = inp("xt", [128, NPAD])
    w1t_d = inp("w1t", [C, C])
    w2t_d = inp("w2t", [C, C])
    wpt_d = inp("wpt", [C, OUT_C])
    b1b_d = inp("b1b", [128, C])
    b2b_d = inp("b2b", [128, C])
    bpb_d = inp("bpb", [128, OUT_C])
    degt_d = inp("degt", [128, NT])
    degl_d = inp("degl", [128, NW])
    iota_d = inp("iota", [128, 128])
    ident_d = inp("ident", [128, 128])
    pmask_d = inp("pmask", [128, 1])
    idxa_d = inp("idxa", [128, max(la16, 16)], I16)
    idxb_d = inp("idxb", [128, max(lb16, 16)], I16)
    rel_d = inp("rel", [128, nchunk])
    out_d = nc.dram_tensor("out", [SP, OUT_C], F32, kind="ExternalOutput").ap()

    g1d = nc.dram_tensor("g1d", [NPAD, C], F32, kind="Internal").ap()
    g2loc = nc.dram_tensor("g2loc", [SP, C], F32, kind="Internal").ap()
    g2d = nc.dram_tensor("g2d", [NPAD, C], F32, kind="Internal",
                         addr_space="Shared").ap()

    XBLK = 512

    from contextlib import ExitStack
    with tile.TileContext(nc) as tc, ExitStack() as ctx:
        cp = ctx.enter_context(tc.tile_pool(name="consts", bufs=1))
        xpool = ctx.enter_context(tc.tile_pool(name="xload", bufs=3))
        gstp = ctx.enter_context(tc.tile_pool(name="gstage", bufs=3))
        msgp = ctx.enter_context(tc.tile_pool(name="msg", bufs=2))
        spool = ctx.enter_context(tc.tile_pool(name="sel", bufs=4))
        epool = ctx.enter_context(tc.tile_pool(name="epi", bufs=4))
        opool = ctx.enter_context(tc.tile_pool(name="otiles", bufs=1))
        ppool_g = ctx.enter_context(tc.tile_pool(name="psg", bufs=2, space="PSUM"))
        ppool_w = ctx.enter_context(tc.tile_pool(name="psw", bufs=2, space="PSUM"))
        ppool_t = ctx.enter_context(tc.tile_pool(name="pst", bufs=2, space="PSUM"))
        ppool_p = ctx.enter_context(tc.tile_pool(name="psp", bufs=2, space="PSUM"))

        def cload(name, ap, shape, dt=F32):
            t = cp.tile(shape, dt, tag=name)
            nc.sync.dma_start(t[:], ap[:])
            return t

        w1t = cload("w1t", w1t_d, [C, C])
        w2t = cload("w2t", w2t_d, [C, C])
        wpt = cload("wpt", wpt_d, [C, OUT_C])
        b1b = cload("b1b", b1b_d, [128, C])
        b2b = cload("b2b", b2b_d, [128, C])
        bpb = cload("bpb", bpb_d, [128, OUT_C])
        degt = cload("degt", degt_d, [128, NT])
        degl = cload("degl", degl_d, [128, NW])
        iota = cload("iota", iota_d, [128, 128])
        ident = cload("ident", ident_d, [128, 128])
        pmask = cload("pmask", pmask_d, [128, 1])
        idxa = cload("idxa", idxa_d, [128, max(la16, 16)], I16)
        idxb = cload("idxb", idxb_d, [128, max(lb16, 16)], I16)
        rel = cload("rel", rel_d, [128, nchunk])

        # dinv = 1/sqrt(deg) (rsqrt activation is banned for accuracy)
        sqf = cp.tile([128, NT], F32, tag="sqf")
        nc.scalar.activation(sqf[:], degt[:], AF.Sqrt)
        dinv = cp.tile([128, NT], F32, tag="dinv")
        nc.vector.reciprocal(dinv[:], sqf[:])
        sql = cp.tile([128, NW], F32, tag="sql")
        nc.scalar.activation(sql[:], degl[:], AF.Sqrt)
        dinvl = cp.tile([128, NW], F32, tag="dinvl")
        nc.vector.reciprocal(dinvl[:], sql[:])

        # ---- phase G1: full table G1 = dinv * (X @ W1.T), node-major ----
        for grp in range(NPAD // XBLK):
            xblk = xpool.tile([128, XBLK], F32, tag="xblk")
            nc.sync.dma_start(xblk[:], xt_d[:, grp * XBLK:(grp + 1) * XBLK])
            gst = gstp.tile([128, XBLK], F32, tag="gst")
            for j in range(XBLK // 128):
                t = grp * (XBLK // 128) + j
                ps = ppool_g.tile([128, C], F32, tag="psg")
                nc.tensor.matmul(ps[:], lhsT=xblk[:, j * 128:(j + 1) * 128],
                                 rhs=w1t[:], start=True, stop=True)
                # alternate PSUM->SBUF scaled copies between DVE and ACT
                dsl = dinv[:, t:t + 1]
                if t % 2 == 0:
                    nc.vector.tensor_scalar_mul(
                        gst[:, j * 128:(j + 1) * 128], ps[:], dsl)
                else:
                    nc.scalar.activation(
                        gst[:, j * 128:(j + 1) * 128], ps[:], AF.Identity,
                        scale=dsl)
            nc.sync.dma_start(
                g1d[grp * XBLK:(grp + 1) * XBLK, :]
                .rearrange("(j p) f -> p j f", p=128),
                gst[:].rearrange("p (j f) -> p j f", f=C))

        # ---- gather + segment-sum windows (shared for both layers) ----
        def window_phase(table, bias_sb, otag):
            halfA = table[0:HALF, :]
            halfB = table[HALF:NPAD, :]
            offa = offb = 0   # in idx columns (16 idx each)
            ci = 0
            otiles = []
            for w in range(NW):
                ps = ppool_w.tile([128, 128], F32, tag="psw")
                nchw = capA[w] + capB[w]
                assert nchw > 0
                done = 0
                GBLK = 7   # dma_gather limit: out free bytes/partition < 4KB
                for half, cap in ((0, capA[w]), (1, capB[w])):
                    for g0 in range(0, cap, GBLK):
                        gb = min(GBLK, cap - g0)
                        nidx = gb * 128
                        if half == 0:
                            isl = idxa[:, offa:offa + nidx // 16]
                            offa += nidx // 16
                            tab = halfA
                        else:
                            isl = idxb[:, offb:offb + nidx // 16]
                            offb += nidx // 16
                            tab = halfB
                        msg = msgp.tile([128, gb, C], F32, tag=f"msg{half}")
                        nc.gpsimd.dma_gather(msg[:], tab, isl, nidx, nidx,
                                             elem_size=C)
                        for k in range(gb):
                            sel = spool.tile([128, 128], F32, tag="sel")
                            nc.vector.tensor_scalar(sel[:], iota[:],
                                                    rel[:, ci:ci + 1], None,
                                                    op0=ALU.is_equal)
                            ci += 1
                            nc.tensor.matmul(ps[:], lhsT=sel[:],
                                             rhs=msg[:, k, :],
                                             start=(done == 0),
                                             stop=(done == nchw - 1))
                            done += 1
                # epilogue: relu(dinv*acc + bias)
                t1 = epool.tile([128, 128], F32, tag="t1")
                nc.vector.tensor_scalar_mul(t1[:], ps[:], dinvl[:, w:w + 1])
                t2 = epool.tile([128, 128], F32, tag="t2")
                nc.vector.tensor_tensor(out=t2[:], in0=t1[:], in1=bias_sb[:],
                                        op=ALU.add)
                o = opool.tile([128, 128], F32, tag=f"{otag}_{w}")
                nc.vector.tensor_scalar_max(o[:], t2[:], 0.0)
                otiles.append(o)
            return otiles

        out1 = window_phase(g1d, b1b, "o1")
        # zero the pad rows of the last tile (they feed G2 through the matmul)
        nc.vector.tensor_scalar_mul(out1[NW - 1][:], out1[NW - 1][:],
                                    pmask[:, 0:1])

        # ---- phase G2: local shard table + AllGather ----
        x2t = cp.tile([128, SP], F32, tag="x2t")
        for w in range(NW):
            pst = ppool_t.tile([128, 128], F32, tag="pst")
            nc.tensor.transpose(pst[:], out1[w][:], ident[:])
            nc.vector.tensor_copy(x2t[:, w * 128:(w + 1) * 128], pst[:])
        for w in range(NW):
            ps = ppool_g.tile([128, C], F32, tag="psg")
            nc.tensor.matmul(ps[:], lhsT=x2t[:, w * 128:(w + 1) * 128],
                             rhs=w2t[:], start=True, stop=True)
            g2t = epool.tile([128, C], F32, tag="g2t")
            if w % 2 == 0:
                nc.vector.tensor_scalar_mul(g2t[:], ps[:], dinvl[:, w:w + 1])
            else:
                nc.scalar.activation(g2t[:], ps[:], AF.Identity,
                                     scale=dinvl[:, w:w + 1])
            nc.sync.dma_start(g2loc[w * 128:(w + 1) * 128, :], g2t[:])
        nc.gpsimd.collective_compute(
            "AllGather", ALU.bypass,
            replica_groups=[list(range(CORES))],
            ins=[g2loc[:]], outs=[g2d[:]])

        out2 = window_phase(g2d, b2b, "o2")

        # ---- projection: relu(h2 @ Wp.T + bp) ----
        for w in range(NW):
            pst = ppool_t.tile([128, 128], F32, tag="pst")
            nc.tensor.transpose(pst[:], out2[w][:], ident[:])
            h2t = epool.tile([128, 128], F32, tag="h2t")
            nc.vector.tensor_copy(h2t[:], pst[:])
            psp = ppool_p.tile([128, OUT_C], F32, tag="psp")
            nc.tensor.matmul(psp[:], lhsT=h2t[:], rhs=wpt[:],
                             start=True, stop=True)
            of = epool.tile([128, OUT_C], F32, tag="of")
            nc.vector.tensor_tensor(out=of[:], in0=psp[:], in1=bpb[:],
                                    op=ALU.add)
            ofr = epool.tile([128, OUT_C], F32, tag="ofr")
            nc.vector.tensor_scalar_max(ofr[:], of[:], 0.0)
            nc.sync.dma_start(out_d[w * 128:(w + 1) * 128, :], ofr[:])

    nc.compile()
    return nc


def _make_in_maps(cfg, sched, shared, per_core, W1, b1, W2, b2, Wp, bp):
    C = cfg.C
    w1t = np.ascontiguousarray(np.asarray(W1, np.float32).T)
    w2t = np.ascontiguousarray(np.asarray(W2, np.float32).T)
    wpt = np.ascontiguousarray(np.asarray(Wp, np.float32).T)
    b1b = np.ascontiguousarray(np.tile(np.asarray(b1, np.float32)[None], (128, 1)))
    b2b = np.ascontiguousarray(np.tile(np.asarray(b2, np.float32)[None], (128, 1)))
    bpb = np.ascontiguousarray(np.tile(np.asarray(bp, np.float32)[None], (128, 1)))
    iota = np.tile(np.arange(128, dtype=np.float32)[None], (128, 1))
    ident = np.eye(128, dtype=np.float32)
    p0 = cfg.S - (cfg.NW - 1) * 128   # real rows in the last window tile
    pmask = np.ascontiguousarray(
        (np.arange(128) < p0).astype(np.float32)[:, None])
    base = dict(xt=shared["xt"], degt=shared["degt"], w1t=w1t, w2t=w2t,
                wpt=wpt, b1b=b1b, b2b=b2b, bpb=bpb, iota=iota, ident=ident,
                pmask=pmask)
    in_maps = []
    for c in range(cfg.CORES):
        pc = per_core[c]
        m = dict(base)
        m["idxa"] = pc["idxa"] if pc["idxa"].size else np.zeros((128, 16), np.int16)
        m["idxb"] = pc["idxb"] if pc["idxb"].size else np.zeros((128, 16), np.int16)
        m["rel"] = pc["rel"]
        m["degl"] = pc["degl"]
        in_maps.append(m)
    return in_maps


def _run(inputs, cfg=None, trace=False, tmpdir=None, verbose=True):
    import time
    t0 = time.time()
    def _log(msg):
        if verbose:
            print(f"[kernel {time.time()-t0:7.1f}s] {msg}", flush=True)
    cfg = cfg or CFG
    sched, shared, per_core = _host_prep(cfg, inputs["x"], inputs["edge_index"])
    _log("host prep done")
    nc = _build_nc(cfg, sched)
    _log("build+compile done")
    in_maps = _make_in_maps(cfg, sched, shared, per_core,
                            inputs["W1"], inputs["b1"], inputs["W2"],
                            inputs["b2"], inputs["Wp"], inputs["bp"])
    _log("in_maps done")
    core_ids = list(range(cfg.CORES))
    if trace:
        # NTFF profiling needs a warm first execute; run once untraced.
        bass_utils.run_bass_kernel_spmd(nc, in_maps, core_ids=core_ids,
                                        trace=False)
        _log("warmup run done")
    res = bass_utils.run_bass_kernel_spmd(nc, in_maps, core_ids=core_ids,
                                          trace=trace, tmpdir=tmpdir)
    _log("run done")
    out = np.empty((cfg.N, cfg.OUT_C), np.float32)
    for c in range(cfg.CORES):
        out[c * cfg.S:(c + 1) * cfg.S] = res.results[c]["out"][:cfg.S]
    return out, res


def kernel(**inputs):
    out, _ = _run(inputs)
    return out
